# revision 26
# baseline (speedup 1.0000x reference)
"""Trainium2 Bass kernel for CustomWindowMHA (sparse window+dilated attention).

Sharding: 8 cores = 2 batches x 4 head-groups (4 heads each). Each core
computes QKV projection for its heads, masked attention, and a partial
output projection against its slice of wo's columns; the host sums the 4
partials per batch.

v2 restructure: the dilated mask (j <= i-132, (i-j)%4 == 0) couples only
tokens with equal residue mod 4, so the dilated part is computed in
phase-grouped coordinates (4 independent 512x512 causal-offset attentions
per head) instead of densely. The 128-wide window band stays in natural
token order (2 j-tiles per q-tile, T0/T1 triangle masks).

v3 (trace-driven):
  - ~30 warmup matmuls on scratch SBUF at t=0 keep the PE HAM
    un-throttled from ~7us (was cold at 1.2GHz until 34us).
  - dilated kj=2,3 score tiles share one PSUM tile -> 3 exp instructions
    per (hp, r, i) instead of 4 (less ACT fixed cost).
  - all mask multiplies moved to gpsimd (was mostly DVE).
  - normalize rebuilt: dilated accumulators flush to bf16 (DVE) and are
    merged into the window PV PSUM by a small identity matmul joining the
    PV accumulation group (replaces a DVE add per head); L rows for all
    4 heads of a q-chunk batch into one [1,2048] row, ONE r4-transpose
    DMA + one [128,16] reciprocal + one DMA back + one gpsimd
    partition_broadcast replace 16 tiny DMAs + 4 ones-matmuls; final
    scale is one scalar_tensor_tensor per head reading poT (PSUM) and
    the broadcast row (SBUF).
  - vaug memsets collapsed to 2 (was 33 x 313ns on gpsimd).
  - PSUM tags: ps512(2) + st(2) + pot(4) = 8 banks.
"""

import sys

sys.path.insert(0, "/opt/trn_rl_repo")

import numpy as np
import ml_dtypes

import concourse.bass as bass
import concourse.mybir as mybir
import concourse.tile as tile
from concourse.vector_clock import ScopedClock
from concourse.bass_utils import run_bass_kernel_spmd

BF16 = mybir.dt.bfloat16
F32 = mybir.dt.float32

B, S, D = 2, 2048, 1024
H, DH = 16, 64
WINDOW, DILATION = 128, 4
P = 128
NT = S // P          # 16 token tiles
KT = D // P          # 8 contraction tiles over D
HPC = 4              # heads per core
QC = 512             # q-chunk width
NQC = S // QC        # 4 q-chunks
NPH = 4              # phases (token residue mod 4)
SP = S // NPH        # 512 tokens per phase
W3 = 3 * HPC * DH    # 768 qkvt columns per k-tile

# mask table column offsets
MT_WA = 0            # window set A composite  [T1 T0 T1 T0]   (512)
MT_WB = 4 * P        # window set B composite  [T0 T1 T0 T1]   (512)
MT_D = 8 * P         # dilated [Bd | Ad]                        (256)
MT_G2 = 10 * P       # dilated g2 composite [Bd | Ad | Bd]      (384)
MT_ID = 13 * P       # identity (65x65 used)                    (128)
MT_W = 14 * P


class _TileContext(tile.TileContext):
    """Kernel-tail Drain gets one wait per live proc, but this walrus build
    allows only a single sync wait on SP Drain — split across drains."""

    def _drain_and_barrier(self, tick_clock, wait_clock):
        drain_inst = self.nc.sync.drain()
        wait_clock.add_sem_waits(
            drain_inst.ins, ScopedClock({None: tick_clock.global_clock})
        )
        si = drain_inst.ins.sync_info
        if si is not None and len(si.on_wait) > 1:
            waits = list(si.on_wait)
            si.on_wait[:] = waits[:1]
            for w in waits[1:]:
                d2 = self.nc.sync.drain()
                si2 = d2.ins.sync_info
                if si2 is None:
                    d2.ins.sync_info = mybir.SyncInfo(on_wait=[w], on_update=[])
                else:
                    si2.on_wait[:] = [w]

        self.nc.all_engine_barrier()
        assert self.sems is not None
        popped = self.nc._tile_sem_poison_stack.pop()
        assert popped is self._sem_poison
        self.nc.clear_and_free_semaphores(list(self.sems.allocated().values()))
        self.nc.all_engine_barrier()


def _split_sync_waits(nc):
    """This walrus build allows only one sync-wait slot on several ISA
    structs. Rewrite the scheduled BIR so every instruction carries at most
    one wait: extra waits move onto same-engine NoOps inserted just before
    (same engine queue => executes in order => semantics preserved)."""
    cnt = 0
    for fn in nc.m.functions:
        for blk in fn.blocks:
            new_insts = []
            for inst in blk.instructions:
                si = inst.sync_info
                if si is not None and si.on_wait and len(si.on_wait) > 1:
                    waits = list(si.on_wait)
                    si.on_wait[:] = waits[-1:]
                    for w in waits[:-1]:
                        cnt += 1
                        nop = mybir.InstNoOp(
                            name=f"waitsplit-{cnt}",
                            engine=inst.engine,
                            ins=[],
                            outs=[],
                            sync_info=mybir.SyncInfo(on_wait=[w], on_update=[]),
                        )
                        new_insts.append(nop)
                new_insts.append(inst)
            blk.instructions[:] = new_insts
    return cnt


def _mask_table() -> np.ndarray:
    """[128, MT_W] bf16 mask table, in ST[j, q] orientation
    (j = partition, q = free):
      T0[sj, sq] = sq >= sj         (window tile (jt, jt))
      T1[sj, sq] = sq <  sj         (window tile (jt, jt+1))
      Bd[skj, sqi] = sqi - skj >= 33   (dilated phase tile qc == kj)
      Ad[skj, sqi] = sqi - skj >= -95  (dilated phase tile qc == kj+1)
    """
    sj = np.arange(P)[:, None]
    sq = np.arange(P)[None, :]
    t0 = (sq >= sj).astype(ml_dtypes.bfloat16)
    t1 = (sq < sj).astype(ml_dtypes.bfloat16)
    bd = ((sq - sj) >= 33).astype(ml_dtypes.bfloat16)
    ad = ((sq - sj) >= -95).astype(ml_dtypes.bfloat16)
    ident = (sq == sj).astype(ml_dtypes.bfloat16)
    out = np.zeros((P, MT_W), dtype=ml_dtypes.bfloat16)
    for k, m in enumerate([t1, t0, t1, t0, t0, t1, t0, t1, bd, ad, bd, ad, bd, ident]):
        out[:, k * P : (k + 1) * P] = m
    return out


def _build_program(repeat: int = 1):
    nc = bass.Bass("TRN2", target_bir_lowering=False, debug=False)

    xt_d = nc.declare_dram_parameter("xt", [D, S], BF16, isOutput=False)
    qkvt_d = nc.declare_dram_parameter("qkvt", [D, W3], BF16, isOutput=False)
    wot_d = nc.declare_dram_parameter("wot", [HPC * DH, D], BF16, isOutput=False)
    mask_d = nc.declare_dram_parameter("mask", [P, MT_W], BF16, isOutput=False)
    y_d = nc.declare_dram_parameter("y", [S, D], F32, isOutput=True)

    with _TileContext(nc) as tc:
        with (
            tc.tile_pool(name="const", bufs=1) as cpool,
            tc.tile_pool(name="work", bufs=2) as wpool,
            tc.tile_pool(name="psum", bufs=2, space="PSUM") as pspool,
        ):
            # ---- persistent SBUF tensors ----
            xt_sb = [cpool.tile([P, S], BF16, tag=f"xt{kt}", name=f"xt{kt}") for kt in range(KT)]
            qkvt_sb = [cpool.tile([P, W3], BF16, tag=f"qkvt{kt}", name=f"qkvt{kt}") for kt in range(KT)]
            mask_sb = cpool.tile([P, MT_W], BF16, tag="mask")
            wot_sb = cpool.tile([P, 2 * D], BF16, tag="wot")
            # natural-order Q^T/K^T per 512-token chunk:
            # cols [mt*QC + t]: mt 0,1 = Q channels 0:128/128:256 (head pairs
            # 0,1); mt 2,3 = K channels. partition = channel within pair.
            qkt_cc = [cpool.tile([P, 4 * QC], BF16, tag=f"qkt{cc}", name=f"qkt{cc}") for cc in range(NQC)]
            # phase-major Q^T staging per head pair: [128 chans, r, qi]
            qphase = [cpool.tile([P, NPH, SP], BF16, tag=f"qph{hp}", name=f"qph{hp}") for hp in range(2)]
            # V (+ones col): one tile each for natural and phase order
            vaug_n = cpool.tile([P, NT, HPC, DH + 1], BF16, tag="vn")
            vaug_p = cpool.tile([P, NPH, NPH, HPC, DH + 1], BF16, tag="vp")
            # dilated accumulators flushed from PSUM: per head [65, r, qi] bf16
            pdacc = [
                cpool.tile([DH + 1, NPH, SP], BF16, tag=f"pd{h}", name=f"pd{h}")
                for h in range(HPC)
            ]
            outt_sb = cpool.tile([P, 2 * S], BF16, tag="outt")  # out^T, ct-major
            scratch = cpool.tile([P, 576], BF16, tag="scr")
            ones1_sb = cpool.tile([1, 64], BF16, tag="ones1")

            nc.gpsimd.memset(ones1_sb[:], 1.0)
            nc.gpsimd.memset(scratch[:], 0.25)
            nc.gpsimd.memset(vaug_n[:], 1.0)
            nc.gpsimd.memset(vaug_p[:], 1.0)

            # ---- PE warmup: keep HAM un-throttled until real matmuls ----
            for _d in range(40):
                ps = pspool.tile([P, QC], F32, tag="st", name=f"warm{_d}")
                nc.tensor.matmul(
                    ps[0:64, :],
                    lhsT=scratch[:, 0:64],
                    rhs=scratch[:, 64:576],
                    start=True,
                    stop=True,
                )

            for _rep in range(repeat):
                # ---- input DMA (per-kt tiles => fine-grained deps).
                # qkvt and the first S-half of xt go first so the (mt, cc<2)
                # projection units are fully fed ~13us in; the second half
                # streams while they compute.
                for kt in range(KT):
                    nc.sync.dma_start(
                        out=qkvt_sb[kt][:], in_=qkvt_d[kt * P : (kt + 1) * P, :]
                    )
                    nc.sync.dma_start(
                        out=xt_sb[kt][:, 0 : S // 2],
                        in_=xt_d[kt * P : (kt + 1) * P, 0 : S // 2],
                    )
                for kt in range(KT):
                    nc.sync.dma_start(
                        out=xt_sb[kt][:, S // 2 : S],
                        in_=xt_d[kt * P : (kt + 1) * P, S // 2 : S],
                    )
                nc.sync.dma_start(out=mask_sb[:], in_=mask_d[:])
                for ct in range(2):
                    nc.sync.dma_start(
                        out=wot_sb[:, ct * D : (ct + 1) * D],
                        in_=wot_d[ct * P : (ct + 1) * P, :],
                    )

                # ---- QT / KT projection (transposed) ----
                # mt 0,1 = Q channels 0:128 / 128:256; mt 2,3 = K channels.
                for mt in range(4):
                    coloff = (0, 128, 256, 384)[mt]
                    for cc in range(NQC):
                        ps = pspool.tile([P, QC], F32, tag="ps512")
                        for kt in range(KT):
                            nc.tensor.matmul(
                                ps[:],
                                lhsT=qkvt_sb[kt][:, coloff : coloff + P],
                                rhs=xt_sb[kt][:, cc * QC : (cc + 1) * QC],
                                start=(kt == 0),
                                stop=(kt == KT - 1),
                            )
                        nc.vector.tensor_copy(
                            qkt_cc[cc][:, mt * QC : (mt + 1) * QC], ps[:]
                        )
                        if mt < 2:
                            # phase-major restage: col 4c+f -> [f, c]
                            nc.vector.tensor_copy(
                                qphase[mt][:, :, cc * P : (cc + 1) * P],
                                ps[:].rearrange("p (c f) -> p f c", f=NPH),
                            )

                # ---- V projection macros (woven into attention below) ----
                def _vproj_nat(nt):
                    def _go():
                        ps = pspool.tile([P, HPC * DH], F32, tag="ps512")
                        for kt in range(KT):
                            nc.tensor.matmul(
                                ps[:],
                                lhsT=xt_sb[kt][:, nt * P : (nt + 1) * P],
                                rhs=qkvt_sb[kt][:, 512:768],
                                start=(kt == 0),
                                stop=(kt == KT - 1),
                            )
                        nc.vector.tensor_copy(
                            vaug_n[:, nt, :, 0:DH],
                            ps[:].rearrange("p (h d) -> p h d", h=HPC),
                        )

                    return _go

                def _vproj_ph(r, kj):
                    def _go():
                        ps = pspool.tile([P, HPC * DH], F32, tag="ps512")
                        for kt in range(KT):
                            nc.tensor.matmul(
                                ps[:],
                                lhsT=xt_sb[kt][:, kj * QC : (kj + 1) * QC].rearrange(
                                    "p (c f) -> p f c", f=NPH
                                )[:, r, :],
                                rhs=qkvt_sb[kt][:, 512:768],
                                start=(kt == 0),
                                stop=(kt == KT - 1),
                            )
                        nc.vector.tensor_copy(
                            vaug_p[:, r, kj, :, 0:DH],
                            ps[:].rearrange("p (h d) -> p h d", h=HPC),
                        )

                    return _go

                # filler queue: PE-heavy macros woven between attention units.
                # Vph pairs (r,0),(r,1) then (r,2),(r,3): pop deadlines match
                # the dilated PV schedule (2 pops/step during hp0).
                fillers = []
                for r in range(NPH):
                    fillers.append(_vproj_ph(r, 0))
                    fillers.append(_vproj_ph(r, 1))
                for r in range(NPH):
                    fillers.append(_vproj_ph(r, 2))
                    fillers.append(_vproj_ph(r, 3))
                for nt in range(NT):
                    fillers.append(_vproj_nat(nt))

                pending = []

                def _flush_pending():
                    while pending:
                        pending.pop(0)()

                # ---------- normalize + wo ----------
                # Per q-chunk (after both head-pair units): dilated bf16
                # accumulators merged into the window PV PSUM by identity
                # matmuls (joined the PV accumulation group upstream), then
                # L rows -> one [1,2048] row -> r4 transpose DMA -> recip ->
                # DMA back -> gpsimd partition broadcast -> per-head stt.
                def _merge_dilated(h, qc, poT):
                    # poT[0:65] += I^T @ pdacc[h] (natural-q interleave view)
                    pdv = pdacc[h][:, :, qc * P : (qc + 1) * P].transpose([0, 2, 1])
                    nc.tensor.matmul(
                        poT[0:65, :],
                        lhsT=mask_sb[0:65, MT_ID : MT_ID + 65],
                        rhs=pdv,
                        start=False,
                        stop=True,
                    )

                def _normalize_qc(qc, poTs4):
                    # poTs4: the 4 heads' merged PSUM tiles (h = 0..3).
                    # Copy merged po (incl. L row 64) to one SBUF tile; this
                    # frees the pot banks early and gives the r4 transpose
                    # DMA a contiguous [1, 2048] L source.
                    po4 = wpool.tile([DH + 1, 4 * QC], F32, tag="po4", bufs=2)
                    for h in range(HPC):
                        if h % 2 == 0:
                            nc.scalar.copy(
                                po4[:, h * QC : (h + 1) * QC], poTs4[h][0:65, :]
                            )
                        else:
                            nc.vector.tensor_copy(
                                po4[:, h * QC : (h + 1) * QC], poTs4[h][0:65, :]
                            )
                    r4 = wpool.tile([P, 16], F32, tag="r4", bufs=2)
                    nc.sync.dma_start(
                        out=r4[:],
                        in_=po4[64:65, :].rearrange("a (p c) -> a p c", p=P),
                    )
                    i4 = wpool.tile([P, 16], BF16, tag="i4", bufs=2)
                    with nc.allow_low_precision("softmax 1/L in bf16"):
                        nc.vector.reciprocal(i4[:], r4[:])
                    invlrow = wpool.tile([1, 4 * QC], BF16, tag="invl", bufs=2)
                    nc.sync.dma_start(
                        out=invlrow[0:1, :].rearrange("a (p c) -> a p c", p=P),
                        in_=i4[:],
                    )

                    def _p2():
                        # Two heads share one PSUM bank (base partitions 0
                        # and 64): 2 pool allocations per chunk instead of
                        # 4, so the ib matmuls no longer ladder through
                        # slot reuse with the stt ops.
                        for j in range(2):
                            ibt = pspool.tile(
                                [P, QC], F32, tag="ps512", name=f"ib{j}", bufs=2
                            )
                            for i in range(2):
                                h = 2 * j + i
                                nc.tensor.matmul(
                                    ibt[64 * i : 64 * i + 64, :],
                                    lhsT=ones1_sb[:],
                                    rhs=invlrow[0:1, h * QC : (h + 1) * QC],
                                    start=True,
                                    stop=True,
                                )
                            for i in range(2):
                                h = 2 * j + i
                                pb = 64 * (h % 2)
                                qoff = (h // 2) * S
                                nc.vector.scalar_tensor_tensor(
                                    out=outt_sb[
                                        pb : pb + 64,
                                        qoff + qc * QC : qoff + (qc + 1) * QC,
                                    ],
                                    in0=po4[0:64, h * QC : (h + 1) * QC],
                                    scalar=1.0,
                                    in1=ibt[64 * i : 64 * i + 64, :],
                                    op0=mybir.AluOpType.mult,
                                    op1=mybir.AluOpType.mult,
                                )

                    pending.append(_p2)

                def _emit_wo(qc):
                    def _go():
                        for qt in range(4 * qc, 4 * qc + 4):
                            ysb = wpool.tile([P, D], F32, tag="ysb", bufs=2)
                            for oc in range(2):
                                yps = pspool.tile([P, QC], F32, tag="ps512")
                                for ct in range(2):
                                    nc.tensor.matmul(
                                        yps[:],
                                        lhsT=outt_sb[
                                            :, ct * S + qt * P : ct * S + (qt + 1) * P
                                        ],
                                        rhs=wot_sb[
                                            :, ct * D + oc * QC : ct * D + (oc + 1) * QC
                                        ],
                                        start=(ct == 0),
                                        stop=(ct == 1),
                                    )
                                if oc == 0:
                                    nc.scalar.copy(
                                        ysb[:, oc * QC : (oc + 1) * QC], yps[:]
                                    )
                                else:
                                    nc.vector.tensor_copy(
                                        ysb[:, oc * QC : (oc + 1) * QC], yps[:]
                                    )
                            nc.sync.dma_start(
                                out=y_d[qt * P : (qt + 1) * P, :], in_=ysb[:]
                            )

                    pending.append(_go)

                # ---------- dilated attention (phase-grouped) ----------
                # per head pair hp, per phase r: three score sub-units:
                #   k0: kj=0 [128, 512]; k1: kj=1 [128, 384];
                #   g2: kj=2 [0:256] + kj=3 [256:384] share one tile.
                # exp on ACT, masks on gpsimd, PV accumulates poTd[65, 512]
                # per (head, phase), flushed to bf16 pdacc by DVE.
                def _run_dilated(hp, pops_per_step, pop_budget):
                    SUBS = [("k0", 512), ("k1", 384), ("g2", 384)]
                    ps_t, e_t = {}, {}
                    poTd = [None, None]

                    # (kj, ps_c0, qi_c0, width) per sub-unit: ps cols are
                    # tile-local, qi cols index the phase row of qphase.
                    ST_PLANS = {
                        "k0": [(0, 0, 0, 512)],
                        "k1": [(1, 0, 128, 384)],
                        "g2": [(2, 0, 256, 256), (3, 256, 384, 128)],
                    }

                    def _issue_st(u):
                        r, sub = u
                        pair = []
                        for i, pb in enumerate((0, 64)):
                            ps = pspool.tile([P, QC], F32, tag="st", name=f"d{sub}")
                            for kj, c0, q0, w in ST_PLANS[sub]:
                                nc.tensor.matmul(
                                    ps[:, c0 : c0 + w],
                                    lhsT=qkt_cc[kj][
                                        pb : pb + 64, (2 + hp) * QC : (3 + hp) * QC
                                    ].rearrange("p (c f) -> p f c", f=NPH)[:, r, :],
                                    rhs=qphase[hp][pb : pb + 64, r, q0 : q0 + w],
                                    start=True,
                                    stop=True,
                                )
                            pair.append(ps)
                        ps_t[u] = pair

                    def _issue_exp(u):
                        r, sub = u
                        n = dict(SUBS)[sub]
                        moff, mw = (MT_D, 256) if sub in ("k0", "k1") else (MT_G2, 384)
                        pair = []
                        for i in range(2):
                            e = wpool.tile([P, n], BF16, tag="e", bufs=8)
                            nc.scalar.activation(
                                e[:],
                                ps_t[u][i][:, 0:n],
                                mybir.ActivationFunctionType.Exp,
                                scale=0.125,
                            )
                            meng = nc.gpsimd if sub == "g2" else nc.vector
                            meng.tensor_mul(
                                e[:, 0:mw], e[:, 0:mw], mask_sb[:, moff : moff + mw]
                            )
                            pair.append(e)
                        del ps_t[u]
                        e_t[u] = pair

                    def _issue_pv(u):
                        r, sub = u
                        if sub == "k0":
                            poTd[0] = pspool.tile([P, SP], F32, tag="pot", name="potd0", bufs=4)
                            poTd[1] = pspool.tile([P, SP], F32, tag="pot", name="potd1", bufs=4)
                        if sub == "k0":
                            plan = [(0, 0, 0, 512)]
                        elif sub == "k1":
                            plan = [(1, 0, 128, 512)]
                        else:
                            plan = [(2, 0, 256, 512), (3, 256, 384, 512)]
                        for i in range(2):
                            for kj, ec, o0, o1 in plan:
                                nc.tensor.matmul(
                                    poTd[i][0:65, o0:o1],
                                    lhsT=vaug_p[:, r, kj, 2 * hp + i, :],
                                    rhs=e_t[u][i][:, ec : ec + (o1 - o0)],
                                    start=(kj == 0),
                                    stop=(kj == NPH - 1),
                                )
                        del e_t[u]
                        if sub == "g2":
                            for i in range(2):
                                nc.vector.tensor_copy(
                                    pdacc[2 * hp + i][:, r, :], poTd[i][0:65, :]
                                )

                    units = [(r, sub) for r in range(NPH) for sub, _ in SUBS]
                    nu = len(units)
                    for step in range(nu + 2):
                        if step < nu:
                            _issue_st(units[step])
                            for _ in range(pops_per_step):
                                if fillers and pop_budget > 0:
                                    pop_budget -= 1
                                    fillers.pop(0)()
                        if 0 <= step - 1 < nu:
                            _issue_exp(units[step - 1])
                        if 0 <= step - 2 < nu:
                            _issue_pv(units[step - 2])

                _run_dilated(0, pops_per_step=2, pop_budget=16)
                _run_dilated(1, pops_per_step=1, pop_budget=12)

                # ---------- window attention + normalize + wo ----------
                # per (qc, hp): two packed score sets:
                #   set A: jts {4qc-1, 4qc+1, 4qc+3} -> psum cols [0:128,
                #          128:384, 384:512] (qc=0 drops jt=-1)
                #   set B: jts {4qc, 4qc+2}          -> psum cols [0:256,
                #          256:512]
                # masks: A -> maskwA [T1 T0 T1 T0], B -> maskwB [T0 T1 T0 T1]
                def _win_st(qc, hp):
                    plans = {
                        "A": [(4 * qc - 1, 0, P), (4 * qc + 1, P, 3 * P),
                              (4 * qc + 3, 3 * P, 4 * P)],
                        "B": [(4 * qc, 0, 2 * P), (4 * qc + 2, 2 * P, 4 * P)],
                    }
                    sets = {}
                    for sk, plan in plans.items():
                        pair = []
                        for i, pb in enumerate((0, 64)):
                            ps = pspool.tile([P, QC], F32, tag="st", name=f"w{sk}")
                            for jt, c0, c1 in plan:
                                if jt < 0:
                                    continue
                                nc.tensor.matmul(
                                    ps[:, c0:c1],
                                    lhsT=qkt_cc[jt // 4][
                                        pb : pb + 64,
                                        (2 + hp) * QC
                                        + (jt % 4) * P : (2 + hp) * QC
                                        + (jt % 4 + 1) * P,
                                    ],
                                    rhs=qkt_cc[qc][
                                        pb : pb + 64, hp * QC + c0 : hp * QC + c1
                                    ],
                                    start=True,
                                    stop=True,
                                )
                            pair.append(ps)
                        sets[sk] = pair
                    return sets

                def _win_expmask(qc, hp, sets):
                    a0 = P if qc == 0 else 0
                    es = {}
                    for sk, moff in (("A", MT_WA), ("B", MT_WB)):
                        c0 = a0 if sk == "A" else 0
                        pair = []
                        for i in range(2):
                            e = wpool.tile([P, QC], BF16, tag="e", bufs=8)
                            nc.scalar.activation(
                                e[:, c0:],
                                sets[sk][i][:, c0:],
                                mybir.ActivationFunctionType.Exp,
                                scale=0.125,
                            )
                            weng = nc.gpsimd if i == 0 else nc.vector
                            weng.tensor_mul(
                                e[:, c0:],
                                e[:, c0:],
                                mask_sb[:, moff + c0 : moff + QC],
                            )
                            pair.append(e)
                        es[sk] = pair
                    return es

                def _win_pv(qc, hp, es, poTs):
                    # One accumulation group per poT bank; the dilated merge
                    # matmul issued right after carries stop=True.
                    plan = [
                        (4 * qc, "B", 0, 2 * P),
                        (4 * qc + 2, "B", 2 * P, 2 * P),
                        (4 * qc + 1, "A", P, 2 * P),
                        (4 * qc + 3, "A", 3 * P, P),
                        (4 * qc - 1, "A", 0, P),
                    ]
                    plan = [p for p in plan if p[0] >= 0]
                    for n_, (jt, sk, ec, w) in enumerate(plan):
                        for i in range(2):
                            nc.tensor.matmul(
                                poTs[i][0:65, ec : ec + w],
                                lhsT=vaug_n[:, jt, 2 * hp + i, :],
                                rhs=es[sk][i][:, ec : ec + w],
                                start=(n_ == 0),
                                stop=False,
                            )

                units = [(qc, hp) for qc in range(NQC) for hp in range(2)]
                nu = len(units)
                stq, eq = {}, {}
                poTs_qc = {}
                for step in range(nu + 2):
                    if step < nu:
                        stq[units[step]] = _win_st(*units[step])
                        if fillers:
                            fillers.pop(0)()
                    if 0 <= step - 1 < nu:
                        u = units[step - 1]
                        eq[u] = _win_expmask(*u, stq.pop(u))
                    if 0 <= step - 2 < nu:
                        u = units[step - 2]
                        qc, hp = u
                        _flush_pending()
                        poTs = [
                            pspool.tile([P, QC], F32, tag="pot", name="potw0", bufs=4),
                            pspool.tile([P, QC], F32, tag="pot", name="potw1", bufs=4),
                        ]
                        _win_pv(qc, hp, eq.pop(u), poTs)
                        _merge_dilated(2 * hp, qc, poTs[0])
                        _merge_dilated(2 * hp + 1, qc, poTs[1])
                        if hp == 0:
                            poTs_qc[qc] = poTs
                        else:
                            _normalize_qc(qc, poTs_qc.pop(qc) + poTs)
                            _emit_wo(qc)
                while fillers:
                    fillers.pop(0)()
                _flush_pending()

    _split_sync_waits(nc)
    return nc


_PROGRAMS = {}


def _program(repeat: int = 1):
    if repeat not in _PROGRAMS:
        _PROGRAMS[repeat] = _build_program(repeat)
    return _PROGRAMS[repeat]


def _prep_inputs(x, qkv, wo):
    """Per-core host-side slicing/transposition/casting."""
    mask = _mask_table()
    in_maps = []
    for c in range(8):
        b, hg = c // 4, c % 4
        h0 = HPC * hg
        rows = np.r_[
            h0 * DH : h0 * DH + HPC * DH,
            D + h0 * DH : D + h0 * DH + HPC * DH,
            2 * D + h0 * DH : 2 * D + h0 * DH + HPC * DH,
        ]
        qkvt = np.ascontiguousarray(qkv[rows].T).astype(ml_dtypes.bfloat16)
        xt = np.ascontiguousarray(x[b].T).astype(ml_dtypes.bfloat16)
        wot = np.ascontiguousarray(
            wo[:, h0 * DH : h0 * DH + HPC * DH].T
        ).astype(ml_dtypes.bfloat16)
        in_maps.append({"xt": xt, "qkvt": qkvt, "wot": wot, "mask": mask})
    return in_maps


def kernel(x, qkv, wo, _trace=False, _trace_kwargs=None):
    x = np.asarray(x, dtype=np.float32)
    qkv = np.asarray(qkv, dtype=np.float32)
    wo = np.asarray(wo, dtype=np.float32)

    nc = _program()
    in_maps = _prep_inputs(x, qkv, wo)
    res = run_bass_kernel_spmd(
        nc, in_maps, list(range(8)), trace=_trace, **(_trace_kwargs or {})
    )
    kernel.last_result = res

    y = np.zeros((B, S, D), dtype=np.float32)
    for c in range(8):
        y[c // 4] += res.results[c]["y"]
    return y


# revision 28
# speedup vs baseline: 1.0238x; 1.0238x over previous
"""Trainium2 Bass kernel for CustomWindowMHA (sparse window+dilated attention).

Sharding: 8 cores = 2 batches x 4 head-groups (4 heads each). Each core
computes QKV projection for its heads, masked attention, and a partial
output projection against its slice of wo's columns; the host sums the 4
partials per batch.

v2 restructure: the dilated mask (j <= i-132, (i-j)%4 == 0) couples only
tokens with equal residue mod 4, so the dilated part is computed in
phase-grouped coordinates (4 independent 512x512 causal-offset attentions
per head) instead of densely. The 128-wide window band stays in natural
token order (2 j-tiles per q-tile, T0/T1 triangle masks).

v3 (trace-driven):
  - ~30 warmup matmuls on scratch SBUF at t=0 keep the PE HAM
    un-throttled from ~7us (was cold at 1.2GHz until 34us).
  - dilated kj=2,3 score tiles share one PSUM tile -> 3 exp instructions
    per (hp, r, i) instead of 4 (less ACT fixed cost).
  - all mask multiplies moved to gpsimd (was mostly DVE).
  - normalize rebuilt: dilated accumulators flush to bf16 (DVE) and are
    merged into the window PV PSUM by a small identity matmul joining the
    PV accumulation group (replaces a DVE add per head); L rows for all
    4 heads of a q-chunk batch into one [1,2048] row, ONE r4-transpose
    DMA + one [128,16] reciprocal + one DMA back + one gpsimd
    partition_broadcast replace 16 tiny DMAs + 4 ones-matmuls; final
    scale is one scalar_tensor_tensor per head reading poT (PSUM) and
    the broadcast row (SBUF).
  - vaug memsets collapsed to 2 (was 33 x 313ns on gpsimd).
  - PSUM tags: ps512(2) + st(2) + pot(4) = 8 banks.
"""

import sys

sys.path.insert(0, "/opt/trn_rl_repo")

import numpy as np
import ml_dtypes

import concourse.bass as bass
import concourse.mybir as mybir
import concourse.tile as tile
from concourse.vector_clock import ScopedClock
from concourse.bass_utils import run_bass_kernel_spmd

BF16 = mybir.dt.bfloat16
F32 = mybir.dt.float32

B, S, D = 2, 2048, 1024
H, DH = 16, 64
WINDOW, DILATION = 128, 4
P = 128
NT = S // P          # 16 token tiles
KT = D // P          # 8 contraction tiles over D
HPC = 4              # heads per core
QC = 512             # q-chunk width
NQC = S // QC        # 4 q-chunks
NPH = 4              # phases (token residue mod 4)
SP = S // NPH        # 512 tokens per phase
W3 = 3 * HPC * DH    # 768 qkvt columns per k-tile

# mask table column offsets
MT_WA = 0            # window set A composite  [T1 T0 T1 T0]   (512)
MT_WB = 4 * P        # window set B composite  [T0 T1 T0 T1]   (512)
MT_D = 8 * P         # dilated [Bd | Ad]                        (256)
MT_G2 = 10 * P       # dilated g2 composite [Bd | Ad | Bd]      (384)
MT_ID = 13 * P       # identity (65x65 used)                    (128)
MT_W = 14 * P


class _TileContext(tile.TileContext):
    """Kernel-tail Drain gets one wait per live proc, but this walrus build
    allows only a single sync wait on SP Drain — split across drains."""

    def _drain_and_barrier(self, tick_clock, wait_clock):
        drain_inst = self.nc.sync.drain()
        wait_clock.add_sem_waits(
            drain_inst.ins, ScopedClock({None: tick_clock.global_clock})
        )
        si = drain_inst.ins.sync_info
        if si is not None and len(si.on_wait) > 1:
            waits = list(si.on_wait)
            si.on_wait[:] = waits[:1]
            for w in waits[1:]:
                d2 = self.nc.sync.drain()
                si2 = d2.ins.sync_info
                if si2 is None:
                    d2.ins.sync_info = mybir.SyncInfo(on_wait=[w], on_update=[])
                else:
                    si2.on_wait[:] = [w]

        self.nc.all_engine_barrier()
        assert self.sems is not None
        popped = self.nc._tile_sem_poison_stack.pop()
        assert popped is self._sem_poison
        self.nc.clear_and_free_semaphores(list(self.sems.allocated().values()))
        self.nc.all_engine_barrier()


def _split_sync_waits(nc):
    """This walrus build allows only one sync-wait slot on several ISA
    structs. Rewrite the scheduled BIR so every instruction carries at most
    one wait: extra waits move onto same-engine NoOps inserted just before
    (same engine queue => executes in order => semantics preserved)."""
    cnt = 0
    for fn in nc.m.functions:
        for blk in fn.blocks:
            new_insts = []
            for inst in blk.instructions:
                si = inst.sync_info
                if si is not None and si.on_wait and len(si.on_wait) > 1:
                    waits = list(si.on_wait)
                    si.on_wait[:] = waits[-1:]
                    for w in waits[:-1]:
                        cnt += 1
                        nop = mybir.InstNoOp(
                            name=f"waitsplit-{cnt}",
                            engine=inst.engine,
                            ins=[],
                            outs=[],
                            sync_info=mybir.SyncInfo(on_wait=[w], on_update=[]),
                        )
                        new_insts.append(nop)
                new_insts.append(inst)
            blk.instructions[:] = new_insts
    return cnt


def _mask_table() -> np.ndarray:
    """[128, MT_W] bf16 mask table, in ST[j, q] orientation
    (j = partition, q = free):
      T0[sj, sq] = sq >= sj         (window tile (jt, jt))
      T1[sj, sq] = sq <  sj         (window tile (jt, jt+1))
      Bd[skj, sqi] = sqi - skj >= 33   (dilated phase tile qc == kj)
      Ad[skj, sqi] = sqi - skj >= -95  (dilated phase tile qc == kj+1)
    """
    sj = np.arange(P)[:, None]
    sq = np.arange(P)[None, :]
    t0 = (sq >= sj).astype(ml_dtypes.bfloat16)
    t1 = (sq < sj).astype(ml_dtypes.bfloat16)
    bd = ((sq - sj) >= 33).astype(ml_dtypes.bfloat16)
    ad = ((sq - sj) >= -95).astype(ml_dtypes.bfloat16)
    ident = (sq == sj).astype(ml_dtypes.bfloat16)
    out = np.zeros((P, MT_W), dtype=ml_dtypes.bfloat16)
    for k, m in enumerate([t1, t0, t1, t0, t0, t1, t0, t1, bd, ad, bd, ad, bd, ident]):
        out[:, k * P : (k + 1) * P] = m
    return out


def _build_program(repeat: int = 1):
    nc = bass.Bass("TRN2", target_bir_lowering=False, debug=False)

    xt_d = nc.declare_dram_parameter("xt", [D, S], BF16, isOutput=False)
    qkvt_d = nc.declare_dram_parameter("qkvt", [D, W3], BF16, isOutput=False)
    wot_d = nc.declare_dram_parameter("wot", [HPC * DH, D], BF16, isOutput=False)
    mask_d = nc.declare_dram_parameter("mask", [P, MT_W], BF16, isOutput=False)
    y_d = nc.declare_dram_parameter("y", [S, D], F32, isOutput=True)

    with _TileContext(nc) as tc:
        with (
            tc.tile_pool(name="const", bufs=1) as cpool,
            tc.tile_pool(name="work", bufs=2) as wpool,
            tc.tile_pool(name="psum", bufs=2, space="PSUM") as pspool,
        ):
            # ---- persistent SBUF tensors ----
            xt_sb = [cpool.tile([P, S], BF16, tag=f"xt{kt}", name=f"xt{kt}") for kt in range(KT)]
            qkvt_sb = [cpool.tile([P, W3], BF16, tag=f"qkvt{kt}", name=f"qkvt{kt}") for kt in range(KT)]
            mask_sb = cpool.tile([P, MT_W], BF16, tag="mask")
            wot_sb = cpool.tile([P, 2 * D], BF16, tag="wot")
            # natural-order Q^T/K^T per 512-token chunk:
            # cols [mt*QC + t]: mt 0,1 = Q channels 0:128/128:256 (head pairs
            # 0,1); mt 2,3 = K channels. partition = channel within pair.
            qkt_cc = [cpool.tile([P, 4 * QC], BF16, tag=f"qkt{cc}", name=f"qkt{cc}") for cc in range(NQC)]
            # phase-major Q^T staging per head pair: [128 chans, r, qi]
            qphase = [cpool.tile([P, NPH, SP], BF16, tag=f"qph{hp}", name=f"qph{hp}") for hp in range(2)]
            # V (+ones col): one tile each for natural and phase order
            vaug_n = cpool.tile([P, NT, HPC, DH + 1], BF16, tag="vn")
            vaug_p = cpool.tile([P, NPH, NPH, HPC, DH + 1], BF16, tag="vp")
            # dilated accumulators flushed from PSUM: per head [65, r, qi] bf16
            pdacc = [
                cpool.tile([DH + 1, NPH, SP], BF16, tag=f"pd{h}", name=f"pd{h}")
                for h in range(HPC)
            ]
            outt_sb = cpool.tile([P, 2 * S], BF16, tag="outt")  # out^T, ct-major
            scratch = cpool.tile([P, 576], BF16, tag="scr")
            ones1_sb = cpool.tile([1, 64], BF16, tag="ones1")

            nc.gpsimd.memset(ones1_sb[:], 1.0)
            nc.gpsimd.memset(scratch[:], 0.25)
            nc.gpsimd.memset(vaug_n[:], 1.0)
            nc.gpsimd.memset(vaug_p[:], 1.0)

            # ---- PE warmup: keep HAM un-throttled until real matmuls ----
            for _d in range(40):
                ps = pspool.tile([P, QC], F32, tag="st", name=f"warm{_d}")
                nc.tensor.matmul(
                    ps[0:64, :],
                    lhsT=scratch[:, 0:64],
                    rhs=scratch[:, 64:576],
                    start=True,
                    stop=True,
                )

            for _rep in range(repeat):
                # ---- input DMA (per-kt tiles => fine-grained deps).
                # qkvt and the first S-half of xt go first so the (mt, cc<2)
                # projection units are fully fed ~13us in; the second half
                # streams while they compute.
                for kt in range(KT):
                    nc.sync.dma_start(
                        out=qkvt_sb[kt][:], in_=qkvt_d[kt * P : (kt + 1) * P, :]
                    )
                    nc.sync.dma_start(
                        out=xt_sb[kt][:, 0 : S // 2],
                        in_=xt_d[kt * P : (kt + 1) * P, 0 : S // 2],
                    )
                for kt in range(KT):
                    nc.sync.dma_start(
                        out=xt_sb[kt][:, S // 2 : S],
                        in_=xt_d[kt * P : (kt + 1) * P, S // 2 : S],
                    )
                nc.sync.dma_start(out=mask_sb[:], in_=mask_d[:])
                for ct in range(2):
                    nc.sync.dma_start(
                        out=wot_sb[:, ct * D : (ct + 1) * D],
                        in_=wot_d[ct * P : (ct + 1) * P, :],
                    )

                # ---- QT / KT projection (transposed) ----
                # mt 0,1 = Q channels 0:128 / 128:256; mt 2,3 = K channels.
                for mt in range(4):
                    coloff = (0, 128, 256, 384)[mt]
                    for cc in range(NQC):
                        ps = pspool.tile([P, QC], F32, tag="ps512")
                        for kt in range(KT):
                            nc.tensor.matmul(
                                ps[:],
                                lhsT=qkvt_sb[kt][:, coloff : coloff + P],
                                rhs=xt_sb[kt][:, cc * QC : (cc + 1) * QC],
                                start=(kt == 0),
                                stop=(kt == KT - 1),
                            )
                        nc.vector.tensor_copy(
                            qkt_cc[cc][:, mt * QC : (mt + 1) * QC], ps[:]
                        )
                        if mt < 2:
                            # phase-major restage: col 4c+f -> [f, c]
                            nc.vector.tensor_copy(
                                qphase[mt][:, :, cc * P : (cc + 1) * P],
                                ps[:].rearrange("p (c f) -> p f c", f=NPH),
                            )

                # ---- V projection macros (woven into attention below) ----
                def _vproj_nat(nt):
                    def _go():
                        ps = pspool.tile([P, HPC * DH], F32, tag="ps512")
                        for kt in range(KT):
                            nc.tensor.matmul(
                                ps[:],
                                lhsT=xt_sb[kt][:, nt * P : (nt + 1) * P],
                                rhs=qkvt_sb[kt][:, 512:768],
                                start=(kt == 0),
                                stop=(kt == KT - 1),
                            )
                        nc.vector.tensor_copy(
                            vaug_n[:, nt, :, 0:DH],
                            ps[:].rearrange("p (h d) -> p h d", h=HPC),
                        )

                    return _go

                def _vproj_ph(r, kj):
                    def _go():
                        ps = pspool.tile([P, HPC * DH], F32, tag="ps512")
                        for kt in range(KT):
                            nc.tensor.matmul(
                                ps[:],
                                lhsT=xt_sb[kt][:, kj * QC : (kj + 1) * QC].rearrange(
                                    "p (c f) -> p f c", f=NPH
                                )[:, r, :],
                                rhs=qkvt_sb[kt][:, 512:768],
                                start=(kt == 0),
                                stop=(kt == KT - 1),
                            )
                        nc.vector.tensor_copy(
                            vaug_p[:, r, kj, :, 0:DH],
                            ps[:].rearrange("p (h d) -> p h d", h=HPC),
                        )

                    return _go

                # filler queue: PE-heavy macros woven between attention units.
                # Vph pairs (r,0),(r,1) then (r,2),(r,3): pop deadlines match
                # the dilated PV schedule (2 pops/step during hp0).
                fillers = []
                for r in range(NPH):
                    fillers.append(_vproj_ph(r, 0))
                    fillers.append(_vproj_ph(r, 1))
                for r in range(NPH):
                    fillers.append(_vproj_ph(r, 2))
                    fillers.append(_vproj_ph(r, 3))
                for nt in range(NT):
                    fillers.append(_vproj_nat(nt))

                pending = []

                def _flush_pending():
                    while pending:
                        pending.pop(0)()

                # ---------- normalize + wo ----------
                # Per q-chunk (after both head-pair units): dilated bf16
                # accumulators merged into the window PV PSUM by identity
                # matmuls (joined the PV accumulation group upstream), then
                # L rows -> one [1,2048] row -> r4 transpose DMA -> recip ->
                # DMA back -> gpsimd partition broadcast -> per-head stt.
                def _merge_dilated(h, qc, poT):
                    # poT[0:65] += I^T @ pdacc[h] (natural-q interleave view)
                    pdv = pdacc[h][:, :, qc * P : (qc + 1) * P].transpose([0, 2, 1])
                    nc.tensor.matmul(
                        poT[0:65, :],
                        lhsT=mask_sb[0:65, MT_ID : MT_ID + 65],
                        rhs=pdv,
                        start=False,
                        stop=True,
                    )

                def _normalize_qc(qc, poTs4):
                    # poTs4: the 4 heads' merged PSUM tiles (h = 0..3).
                    # Copy merged po (incl. L row 64) to one SBUF tile; this
                    # frees the pot banks early and gives the r4 transpose
                    # DMA a contiguous [1, 2048] L source.
                    po4 = wpool.tile([DH + 1, 4 * QC], F32, tag="po4", bufs=3)
                    for h in range(HPC):
                        if h % 2 == 0:
                            nc.scalar.copy(
                                po4[:, h * QC : (h + 1) * QC], poTs4[h][0:65, :]
                            )
                        else:
                            nc.vector.tensor_copy(
                                po4[:, h * QC : (h + 1) * QC], poTs4[h][0:65, :]
                            )
                    r4 = wpool.tile([P, 16], F32, tag="r4", bufs=3)
                    nc.sync.dma_start(
                        out=r4[:],
                        in_=po4[64:65, :].rearrange("a (p c) -> a p c", p=P),
                    )
                    i4 = wpool.tile([P, 16], BF16, tag="i4", bufs=3)
                    with nc.allow_low_precision("softmax 1/L in bf16"):
                        nc.vector.reciprocal(i4[:], r4[:])
                    invlrow = wpool.tile([1, 4 * QC], BF16, tag="invl", bufs=3)
                    nc.sync.dma_start(
                        out=invlrow[0:1, :].rearrange("a (p c) -> a p c", p=P),
                        in_=i4[:],
                    )

                    def _p2():
                        # Two heads share one PSUM bank (base partitions 0
                        # and 64): 2 pool allocations per chunk instead of
                        # 4, so the ib matmuls no longer ladder through
                        # slot reuse with the stt ops.
                        for j in range(2):
                            ibt = pspool.tile(
                                [P, QC], F32, tag="ps512", name=f"ib{j}", bufs=2
                            )
                            for i in range(2):
                                h = 2 * j + i
                                nc.tensor.matmul(
                                    ibt[64 * i : 64 * i + 64, :],
                                    lhsT=ones1_sb[:],
                                    rhs=invlrow[0:1, h * QC : (h + 1) * QC],
                                    start=True,
                                    stop=True,
                                )
                            for i in range(2):
                                h = 2 * j + i
                                pb = 64 * (h % 2)
                                qoff = (h // 2) * S
                                nc.vector.scalar_tensor_tensor(
                                    out=outt_sb[
                                        pb : pb + 64,
                                        qoff + qc * QC : qoff + (qc + 1) * QC,
                                    ],
                                    in0=po4[0:64, h * QC : (h + 1) * QC],
                                    scalar=1.0,
                                    in1=ibt[64 * i : 64 * i + 64, :],
                                    op0=mybir.AluOpType.mult,
                                    op1=mybir.AluOpType.mult,
                                )

                    pending.append(_p2)

                def _emit_wo(qc):
                    def _go():
                        for qt in range(4 * qc, 4 * qc + 4):
                            ysb = wpool.tile([P, D], F32, tag="ysb", bufs=4)
                            for oc in range(2):
                                yps = pspool.tile([P, QC], F32, tag="ps512")
                                for ct in range(2):
                                    nc.tensor.matmul(
                                        yps[:],
                                        lhsT=outt_sb[
                                            :, ct * S + qt * P : ct * S + (qt + 1) * P
                                        ],
                                        rhs=wot_sb[
                                            :, ct * D + oc * QC : ct * D + (oc + 1) * QC
                                        ],
                                        start=(ct == 0),
                                        stop=(ct == 1),
                                    )
                                if oc == 0:
                                    nc.scalar.copy(
                                        ysb[:, oc * QC : (oc + 1) * QC], yps[:]
                                    )
                                else:
                                    nc.vector.tensor_copy(
                                        ysb[:, oc * QC : (oc + 1) * QC], yps[:]
                                    )
                            nc.sync.dma_start(
                                out=y_d[qt * P : (qt + 1) * P, :], in_=ysb[:]
                            )

                    pending.append(_go)

                # ---------- dilated attention (phase-grouped) ----------
                # per head pair hp, per phase r: three score sub-units:
                #   k0: kj=0 [128, 512]; k1: kj=1 [128, 384];
                #   g2: kj=2 [0:256] + kj=3 [256:384] share one tile.
                # exp on ACT, masks on gpsimd, PV accumulates poTd[65, 512]
                # per (head, phase), flushed to bf16 pdacc by DVE.
                def _run_dilated(hp, pops_per_step, pop_budget):
                    SUBS = [("k0", 512), ("k1", 384), ("g2", 384)]
                    ps_t, e_t = {}, {}
                    poTd = [None, None]

                    # (kj, ps_c0, qi_c0, width) per sub-unit: ps cols are
                    # tile-local, qi cols index the phase row of qphase.
                    ST_PLANS = {
                        "k0": [(0, 0, 0, 512)],
                        "k1": [(1, 0, 128, 384)],
                        "g2": [(2, 0, 256, 256), (3, 256, 384, 128)],
                    }

                    def _issue_st(u):
                        r, sub = u
                        pair = []
                        for i, pb in enumerate((0, 64)):
                            ps = pspool.tile([P, QC], F32, tag="st", name=f"d{sub}")
                            for kj, c0, q0, w in ST_PLANS[sub]:
                                nc.tensor.matmul(
                                    ps[:, c0 : c0 + w],
                                    lhsT=qkt_cc[kj][
                                        pb : pb + 64, (2 + hp) * QC : (3 + hp) * QC
                                    ].rearrange("p (c f) -> p f c", f=NPH)[:, r, :],
                                    rhs=qphase[hp][pb : pb + 64, r, q0 : q0 + w],
                                    start=True,
                                    stop=True,
                                )
                            pair.append(ps)
                        ps_t[u] = pair

                    def _issue_exp(u):
                        r, sub = u
                        n = dict(SUBS)[sub]
                        moff, mw = (MT_D, 256) if sub in ("k0", "k1") else (MT_G2, 384)
                        pair = []
                        for i in range(2):
                            e = wpool.tile([P, n], BF16, tag="e", bufs=12)
                            nc.scalar.activation(
                                e[:],
                                ps_t[u][i][:, 0:n],
                                mybir.ActivationFunctionType.Exp,
                                scale=0.125,
                            )
                            meng = nc.gpsimd if sub == "g2" else nc.vector
                            meng.tensor_mul(
                                e[:, 0:mw], e[:, 0:mw], mask_sb[:, moff : moff + mw]
                            )
                            pair.append(e)
                        del ps_t[u]
                        e_t[u] = pair

                    def _issue_pv(u):
                        r, sub = u
                        if sub == "k0":
                            poTd[0] = pspool.tile([P, SP], F32, tag="pot", name="potd0", bufs=4)
                            poTd[1] = pspool.tile([P, SP], F32, tag="pot", name="potd1", bufs=4)
                        if sub == "k0":
                            plan = [(0, 0, 0, 512)]
                        elif sub == "k1":
                            plan = [(1, 0, 128, 512)]
                        else:
                            plan = [(2, 0, 256, 512), (3, 256, 384, 512)]
                        for i in range(2):
                            for kj, ec, o0, o1 in plan:
                                nc.tensor.matmul(
                                    poTd[i][0:65, o0:o1],
                                    lhsT=vaug_p[:, r, kj, 2 * hp + i, :],
                                    rhs=e_t[u][i][:, ec : ec + (o1 - o0)],
                                    start=(kj == 0),
                                    stop=(kj == NPH - 1),
                                )
                        del e_t[u]
                        if sub == "g2":
                            for i in range(2):
                                nc.vector.tensor_copy(
                                    pdacc[2 * hp + i][:, r, :], poTd[i][0:65, :]
                                )

                    units = [(r, sub) for r in range(NPH) for sub, _ in SUBS]
                    nu = len(units)
                    for step in range(nu + 2):
                        if step < nu:
                            _issue_st(units[step])
                            for _ in range(pops_per_step):
                                if fillers and pop_budget > 0:
                                    pop_budget -= 1
                                    fillers.pop(0)()
                        if 0 <= step - 1 < nu:
                            _issue_exp(units[step - 1])
                        if 0 <= step - 2 < nu:
                            _issue_pv(units[step - 2])

                _run_dilated(0, pops_per_step=2, pop_budget=16)
                _run_dilated(1, pops_per_step=1, pop_budget=12)

                # ---------- window attention + normalize + wo ----------
                # per (qc, hp): two packed score sets:
                #   set A: jts {4qc-1, 4qc+1, 4qc+3} -> psum cols [0:128,
                #          128:384, 384:512] (qc=0 drops jt=-1)
                #   set B: jts {4qc, 4qc+2}          -> psum cols [0:256,
                #          256:512]
                # masks: A -> maskwA [T1 T0 T1 T0], B -> maskwB [T0 T1 T0 T1]
                def _win_st(qc, hp):
                    plans = {
                        "A": [(4 * qc - 1, 0, P), (4 * qc + 1, P, 3 * P),
                              (4 * qc + 3, 3 * P, 4 * P)],
                        "B": [(4 * qc, 0, 2 * P), (4 * qc + 2, 2 * P, 4 * P)],
                    }
                    sets = {}
                    for sk, plan in plans.items():
                        pair = []
                        for i, pb in enumerate((0, 64)):
                            ps = pspool.tile([P, QC], F32, tag="st", name=f"w{sk}")
                            for jt, c0, c1 in plan:
                                if jt < 0:
                                    continue
                                nc.tensor.matmul(
                                    ps[:, c0:c1],
                                    lhsT=qkt_cc[jt // 4][
                                        pb : pb + 64,
                                        (2 + hp) * QC
                                        + (jt % 4) * P : (2 + hp) * QC
                                        + (jt % 4 + 1) * P,
                                    ],
                                    rhs=qkt_cc[qc][
                                        pb : pb + 64, hp * QC + c0 : hp * QC + c1
                                    ],
                                    start=True,
                                    stop=True,
                                )
                            pair.append(ps)
                        sets[sk] = pair
                    return sets

                def _win_expmask(qc, hp, sets):
                    a0 = P if qc == 0 else 0
                    es = {}
                    for sk, moff in (("A", MT_WA), ("B", MT_WB)):
                        c0 = a0 if sk == "A" else 0
                        pair = []
                        for i in range(2):
                            e = wpool.tile([P, QC], BF16, tag="e", bufs=12)
                            nc.scalar.activation(
                                e[:, c0:],
                                sets[sk][i][:, c0:],
                                mybir.ActivationFunctionType.Exp,
                                scale=0.125,
                            )
                            nc.vector.tensor_mul(
                                e[:, c0:],
                                e[:, c0:],
                                mask_sb[:, moff + c0 : moff + QC],
                            )
                            pair.append(e)
                        es[sk] = pair
                    return es

                def _win_pv(qc, hp, es, poTs):
                    # One accumulation group per poT bank; the dilated merge
                    # matmul issued right after carries stop=True.
                    plan = [
                        (4 * qc, "B", 0, 2 * P),
                        (4 * qc + 2, "B", 2 * P, 2 * P),
                        (4 * qc + 1, "A", P, 2 * P),
                        (4 * qc + 3, "A", 3 * P, P),
                        (4 * qc - 1, "A", 0, P),
                    ]
                    plan = [p for p in plan if p[0] >= 0]
                    for n_, (jt, sk, ec, w) in enumerate(plan):
                        for i in range(2):
                            nc.tensor.matmul(
                                poTs[i][0:65, ec : ec + w],
                                lhsT=vaug_n[:, jt, 2 * hp + i, :],
                                rhs=es[sk][i][:, ec : ec + w],
                                start=(n_ == 0),
                                stop=False,
                            )

                units = [(qc, hp) for qc in range(NQC) for hp in range(2)]
                nu = len(units)
                stq, eq = {}, {}
                poTs_qc = {}
                for step in range(nu + 2):
                    if step < nu:
                        stq[units[step]] = _win_st(*units[step])
                        if fillers:
                            fillers.pop(0)()
                    if 0 <= step - 1 < nu:
                        u = units[step - 1]
                        eq[u] = _win_expmask(*u, stq.pop(u))
                    if 0 <= step - 2 < nu:
                        u = units[step - 2]
                        qc, hp = u
                        _flush_pending()
                        poTs = [
                            pspool.tile([P, QC], F32, tag="pot", name="potw0", bufs=4),
                            pspool.tile([P, QC], F32, tag="pot", name="potw1", bufs=4),
                        ]
                        _win_pv(qc, hp, eq.pop(u), poTs)
                        _merge_dilated(2 * hp, qc, poTs[0])
                        _merge_dilated(2 * hp + 1, qc, poTs[1])
                        if hp == 0:
                            poTs_qc[qc] = poTs
                        else:
                            _normalize_qc(qc, poTs_qc.pop(qc) + poTs)
                            _emit_wo(qc)
                while fillers:
                    fillers.pop(0)()
                _flush_pending()

    _split_sync_waits(nc)
    return nc


_PROGRAMS = {}


def _program(repeat: int = 1):
    if repeat not in _PROGRAMS:
        _PROGRAMS[repeat] = _build_program(repeat)
    return _PROGRAMS[repeat]


def _prep_inputs(x, qkv, wo):
    """Per-core host-side slicing/transposition/casting."""
    mask = _mask_table()
    in_maps = []
    for c in range(8):
        b, hg = c // 4, c % 4
        h0 = HPC * hg
        rows = np.r_[
            h0 * DH : h0 * DH + HPC * DH,
            D + h0 * DH : D + h0 * DH + HPC * DH,
            2 * D + h0 * DH : 2 * D + h0 * DH + HPC * DH,
        ]
        qkvt = np.ascontiguousarray(qkv[rows].T).astype(ml_dtypes.bfloat16)
        xt = np.ascontiguousarray(x[b].T).astype(ml_dtypes.bfloat16)
        wot = np.ascontiguousarray(
            wo[:, h0 * DH : h0 * DH + HPC * DH].T
        ).astype(ml_dtypes.bfloat16)
        in_maps.append({"xt": xt, "qkvt": qkvt, "wot": wot, "mask": mask})
    return in_maps


def kernel(x, qkv, wo, _trace=False, _trace_kwargs=None):
    x = np.asarray(x, dtype=np.float32)
    qkv = np.asarray(qkv, dtype=np.float32)
    wo = np.asarray(wo, dtype=np.float32)

    nc = _program()
    in_maps = _prep_inputs(x, qkv, wo)
    res = run_bass_kernel_spmd(
        nc, in_maps, list(range(8)), trace=_trace, **(_trace_kwargs or {})
    )
    kernel.last_result = res

    y = np.zeros((B, S, D), dtype=np.float32)
    for c in range(8):
        y[c // 4] += res.results[c]["y"]
    return y
